# revision 21
# baseline (speedup 1.0000x reference)
"""BEV camera-to-grid scatter kernel for Trainium2 (8 NeuronCores).

Strategy (BEVPoolV2-style):
 - Host (planning only): conservatively cull the 2M frustum points with f64
   geometry + margin (16.6% survive), sort kept points of each camera along a
   Morton curve of their target BEV cell, split contiguously across the 8
   cores, and pack into 1408-point blocks whose (margin-padded) cell windows
   fit a uniform WXP x WYP class. Ship features as bf16 plus small f32 tables
   (pixel coords, depth, per-block affine coefs, per-block exact f32 bin-edge
   thresholds).
 - Device (one uniform SPMD program, no control flow): batched f32 geometry
   for all points (exact reference op structure), exact binning via threshold
   compares diffed into per-axis one-hot indicators Ax/Ay, per-block one-hot
   outer products, bf16 matmul scatter into per-block PSUM windows, results
   appended to a per-block slots buffer, one DMA out. Cores are pure data
   parallel - no cross-core communication on device.
 - Host (unshard): place each block's window into the full (mostly zero) BEV
   grid and sum across cores (scatter-add is associative).
"""
import sys
import numpy as np

sys.path.insert(0, '/opt/trn_rl_repo')
import ml_dtypes

B, N, D, FH, FW, C = 1, 6, 118, 32, 88, 80
IH, IW = 256, 704
NX, NY, NZ = 360, 360, 1
DXS = (0.3, 0.3, 20.0)
COFF = (-54.0, -54.0, -10.0)   # exact f32 of reference's (bx - dx/2)
NCORES = 8
BLK = 1408                     # points per block: 128 partitions x 11 cols
UJ = 11
WXP, WYP = 8, 6                # uniform per-COLUMN window class (128 points)
WP = WXP * WYP
BWP = UJ * WP                  # slot cells per block
MARGIN_Q = 0.02                # conservative cull margin, in cell units
NCO = 24                       # per-block coefs: A(9) b(3) M(9) t(3)
PADTHR = 3.0e38
f32 = np.float32


# ---------------------------------------------------------------- thresholds
def _thresholds():
    """Exact f32 cell-edge thresholds replicating trunc((g-coff)/dx) binning.

    L[k] = smallest f32 g with q_of(g) >= k (k>=1); L[0] uses q_of(g) > -1
    (reference: trunc + coords>=0 keeps q in (-1,0) in bin 0).
    """
    out = []
    for ax, nb in ((0, NX), (1, NY), (2, NZ)):
        coff = f32(COFF[ax]); dx = f32(DXS[ax])

        def q_of(g):
            return f32(f32(f32(g) - coff) / dx)

        def smallest(pred, lo, hi):
            def key(i):
                return np.int64(i) if i >= 0 else np.int64(-2147483648) - np.int64(i)

            def unkey(k):
                return np.int32(k) if k >= 0 else np.int32(-(k + 2147483648))

            kl = key(f32(lo).view(np.int32)); kh = key(f32(hi).view(np.int32))
            assert not pred(unkey(kl).view(f32)) and pred(unkey(kh).view(f32))
            while kh - kl > 1:
                km = (kl + kh) // 2
                if pred(unkey(km).view(f32)):
                    kh = km
                else:
                    kl = km
            return unkey(kh).view(f32)

        lo_p = f32(coff - 4 * dx); hi_p = f32(coff + (nb + 4) * dx)
        L = np.empty(nb + 1, f32)
        L[0] = smallest(lambda g: q_of(g) > f32(-1.0), lo_p, hi_p)
        for k in range(1, nb + 1):
            L[k] = smallest(lambda g, k=k: q_of(g) >= f32(k), lo_p, hi_p)
        out.append(L)
    return out


_THR_CACHE = []


def _get_thresholds():
    if not _THR_CACHE:
        _THR_CACHE.append(_thresholds())
    return _THR_CACHE[0]


# ------------------------------------------------------------------- planning
def _frustum_axes():
    ds = np.arange(1.0, 60.0, 0.5, dtype=f32)
    xs = np.linspace(0.0, IW - 1, FW, dtype=f32)
    ys = np.linspace(0.0, IH - 1, FH, dtype=f32)
    return ds, xs, ys


def _compute_coeffs(inputs):
    """Fold the reference chain into per-cam affine A,b (pixel->p0) and M,t."""
    aug = np.asarray(inputs['img_aug_matrix'], np.float64)
    c2e = np.asarray(inputs['camera2ego'], np.float64)
    intr = np.asarray(inputs['camera_intrinsics'], np.float64)
    l2e = np.asarray(inputs['lidar2ego'], np.float64)
    laug = np.asarray(inputs['lidar_aug_matrix'], np.float64)
    inv_pr = np.linalg.inv(aug[..., :3, :3])
    post_trans = aug[..., :3, 3]
    A64 = inv_pr
    b64 = -np.einsum('bnij,bnj->bni', inv_pr, post_trans)
    combine = c2e[..., :3, :3] @ np.linalg.inv(intr[..., :3, :3])
    pre = laug[..., :3, :3] @ np.linalg.inv(l2e[..., :3, :3])
    M64 = np.einsum('bij,bnjk->bnik', pre, combine)
    t64 = np.einsum('bij,bnj->bni', pre, c2e[..., :3, 3] - l2e[..., :3, 3][:, None, :]) \
        + laug[..., :3, 3][:, None, :]
    return A64[0], b64[0], M64[0], t64[0]


def _geom64(A, b, M, t, px, py, dv):
    p0 = [A[k, 0] * px + A[k, 1] * py + (A[k, 2] * dv + b[k]) for k in range(3)]
    uu = p0[0] * p0[2]
    vv = p0[1] * p0[2]
    return [(uu * M[k, 0] + vv * M[k, 1]) + p0[2] * M[k, 2] + t[k] for k in range(3)]


def _build_plan(inputs):
    A64, b64, M64, t64 = _compute_coeffs(inputs)
    ds, xs, ys = _frustum_axes()
    dvg, pyg, pxg = np.meshgrid(ds.astype(np.float64), ys.astype(np.float64),
                                xs.astype(np.float64), indexing='ij')
    pxg = np.ascontiguousarray(pxg.ravel())
    pyg = np.ascontiguousarray(pyg.ravel())
    dvg = np.ascontiguousarray(dvg.ravel())
    colslists = [[] for _ in range(NCORES)]
    for n in range(N):
        gx, gy, gz = _geom64(A64[n], b64[n], M64[n], t64[n], pxg, pyg, dvg)
        qx = (gx - COFF[0]) / DXS[0]
        qy = (gy - COFF[1]) / DXS[1]
        qz = (gz - COFF[2]) / DXS[2]
        m = MARGIN_Q
        keep = ((qx > -1 - m) & (qx < NX + m) &
                (qy > -1 - m) & (qy < NY + m) &
                (qz > -1 - m) & (qz < NZ + m))
        idx = np.nonzero(keep)[0]
        kx = np.maximum(np.floor(qx[idx]), 0).astype(np.int64)
        ky = np.maximum(np.floor(qy[idx]), 0).astype(np.int64)
        code = np.zeros(len(kx), np.int64)
        for bit in range(9):
            code |= ((kx >> bit) & 1) << (2 * bit) | ((ky >> bit) & 1) << (2 * bit + 1)
        order = np.argsort(code, kind='stable')
        idx, kx, ky = idx[order], kx[order], ky[order]
        K = len(idx)
        bounds = [K * c // NCORES for c in range(NCORES + 1)]
        for c in range(NCORES):
            lo, hi = bounds[c], bounds[c + 1]
            i = lo
            cols = []
            while i < hi:
                j = i
                x0 = x1 = kx[i]; y0 = y1 = ky[i]
                while j < hi and j - i < 128:
                    nx0 = min(x0, kx[j]); nx1 = max(x1, kx[j])
                    ny0 = min(y0, ky[j]); ny1 = max(y1, ky[j])
                    if nx1 - nx0 + 3 > WXP or ny1 - ny0 + 3 > WYP:
                        break
                    x0, x1, y0, y1 = nx0, nx1, ny0, ny1
                    j += 1
                cols.append(dict(idx=idx[i:j], cam=n,
                                 kx0=max(int(x0) - 1, 0),
                                 ky0=max(int(y0) - 1, 0)))
                i = j
            colslists[c].extend(cols)
    # group 11 columns per block (cameras may mix: coefs are per column)
    cores = [[dict(cols=cl[b0:b0 + UJ]) for b0 in range(0, len(cl), UJ)]
             for cl in colslists]
    NBC = max(len(c) for c in cores)
    return dict(A64=A64, b64=b64, M64=M64, t64=t64, cores=cores, NBC=NBC,
                pxg=pxg, pyg=pyg, dvg=dvg)


def _pack_core(plan, inputs, c):
    """Device-side tables for core c."""
    Lx, Ly, Lz = _get_thresholds()
    NBC = plan['NBC']
    cf = np.asarray(inputs['cam_feats'], f32)[0].reshape(N, -1, C)
    blocks = plan['cores'][c]
    feats = np.zeros((NBC, BLK, C), ml_dtypes.bfloat16)
    pxt = np.zeros((128, UJ * NBC), f32)
    pyt = np.zeros((128, UJ * NBC), f32)
    dvt = np.zeros((128, UJ * NBC), f32)
    coef = np.zeros((NBC, UJ, NCO), f32)
    thrx = np.full((NBC, UJ, WXP + 1), PADTHR, f32)
    thry = np.full((NBC, UJ, WYP + 1), PADTHR, f32)
    for s, blk in enumerate(blocks):
        fe = feats[s].reshape(128, UJ, C)
        for j, col in enumerate(blk['cols']):
            n = col['cam']
            A = plan['A64'][n].astype(f32); b = plan['b64'][n].astype(f32)
            M = plan['M64'][n].astype(f32); t = plan['t64'][n].astype(f32)
            coef[s, j] = np.array(list(A.ravel()) + list(b) + list(M.ravel())
                                  + list(t), f32)
            idx = col['idx']
            k = len(idx)
            fe[:k, j] = cf[n][idx].astype(ml_dtypes.bfloat16)
            pxt[:k, s * UJ + j] = plan['pxg'][idx]
            pyt[:k, s * UJ + j] = plan['pyg'][idx]
            dvt[:k, s * UJ + j] = plan['dvg'][idx]
            ex = min(WXP + 1, NX + 1 - col['kx0'])
            ey = min(WYP + 1, NY + 1 - col['ky0'])
            thrx[s, j, :ex] = Lx[col['kx0']:col['kx0'] + ex]
            thry[s, j, :ey] = Ly[col['ky0']:col['ky0'] + ey]
    coefa = np.ascontiguousarray(coef[..., :12]).reshape(1, NBC * UJ * 12)
    coefm = np.ascontiguousarray(coef[..., 12:]).reshape(1, NBC * UJ * 12)
    thrxb = thrx.reshape(1, NBC * UJ * (WXP + 1)).copy()
    thryb = thry.reshape(1, NBC * UJ * (WYP + 1)).copy()
    return dict(feats=feats, pxt=pxt, pyt=pyt, dvt=dvt, coefa=coefa,
                coefm=coefm, thrx=thrxb, thry=thryb)


# ----------------------------------------------------------------- bass build
def _build_bass(NBC):
    import concourse.bacc as bacc
    import concourse.mybir as mybir
    import concourse.tile as tile

    SJ = NBC * UJ
    f32t = mybir.dt.float32
    bf16 = mybir.dt.bfloat16
    AL = mybir.AluOpType
    Lx, Ly, Lz = _get_thresholds()
    LZ0, LZ1 = float(Lz[0]), float(Lz[1])

    nc = bacc.Bacc(None, target_bir_lowering=False, num_devices=NCORES)
    feats_t = nc.dram_tensor("feats", [NBC, BLK, C], bf16, kind="ExternalInput")
    pxt_t = nc.dram_tensor("pxt", [128, SJ], f32t, kind="ExternalInput")
    pyt_t = nc.dram_tensor("pyt", [128, SJ], f32t, kind="ExternalInput")
    dvt_t = nc.dram_tensor("dvt", [128, SJ], f32t, kind="ExternalInput")
    coefa_t = nc.dram_tensor("coefa", [1, NBC * UJ * 12], f32t, kind="ExternalInput")
    coefm_t = nc.dram_tensor("coefm", [1, NBC * UJ * 12], f32t, kind="ExternalInput")
    thrx_t = nc.dram_tensor("thrx", [1, NBC * UJ * (WXP + 1)], f32t,
                            kind="ExternalInput")
    thry_t = nc.dram_tensor("thry", [1, NBC * UJ * (WYP + 1)], f32t,
                            kind="ExternalInput")
    slots_t = nc.dram_tensor("slots", [C, NBC * BWP], bf16, kind="ExternalOutput")

    with tile.TileContext(nc) as tc:
        with tc.tile_pool(name="tabs", bufs=1) as tp, \
             tc.tile_pool(name="fb", bufs=2) as fp, \
             tc.tile_pool(name="oh", bufs=4) as op_, \
             tc.tile_pool(name="ps", bufs=4, space="PSUM") as pp:

            pxt = tp.tile([128, SJ], f32t); nc.sync.dma_start(pxt[:], pxt_t[:])
            pyt = tp.tile([128, SJ], f32t); nc.sync.dma_start(pyt[:], pyt_t[:])
            dvt = tp.tile([128, SJ], f32t); nc.sync.dma_start(dvt[:], dvt_t[:])
            coefa1 = tp.tile([1, NBC * UJ * 12], f32t)
            nc.sync.dma_start(coefa1[:], coefa_t[:])
            coefm1 = tp.tile([1, NBC * UJ * 12], f32t)
            nc.sync.dma_start(coefm1[:], coefm_t[:])
            thrx1 = tp.tile([1, NBC * UJ * (WXP + 1)], f32t)
            nc.sync.dma_start(thrx1[:], thrx_t[:])
            thry1 = tp.tile([1, NBC * UJ * (WYP + 1)], f32t)
            nc.sync.dma_start(thry1[:], thry_t[:])
            cofa = tp.tile([128, NBC * UJ * 12], f32t, tag="coeftab", name="cofa")
            nc.gpsimd.partition_broadcast(cofa[:], coefa1[:])
            thrx = tp.tile([128, NBC * UJ * (WXP + 1)], f32t)
            nc.gpsimd.partition_broadcast(thrx[:], thrx1[:])
            thry = tp.tile([128, NBC * UJ * (WYP + 1)], f32t)
            nc.gpsimd.partition_broadcast(thry[:], thry1[:])

            coefa4 = cofa[:].rearrange("p (s j k) -> p s j k", j=UJ, k=12)

            def cslice(kidx):
                return coefa4[:, :, :, kidx]

            def g3(ap):
                return ap.rearrange("p (s j) -> p s j", j=UJ)

            # ---- batched geometry, exact f32 op order ----
            tmpa = tp.tile([128, SJ], f32t)
            tmpb = tp.tile([128, SJ], f32t)
            p0 = [tp.tile([128, SJ], f32t, name=f'p0_{i}', tag=f'p0_{i}')
                  for i in range(3)]
            for kk in range(3):
                nc.vector.tensor_tensor(out=g3(tmpa[:]), in0=g3(pxt[:]),
                                        in1=cslice(3 * kk + 0), op=AL.mult)
                nc.vector.tensor_tensor(out=g3(tmpb[:]), in0=g3(pyt[:]),
                                        in1=cslice(3 * kk + 1), op=AL.mult)
                nc.vector.tensor_tensor(out=tmpa[:], in0=tmpa[:], in1=tmpb[:], op=AL.add)
                nc.vector.tensor_tensor(out=g3(tmpb[:]), in0=g3(dvt[:]),
                                        in1=cslice(3 * kk + 2), op=AL.mult)
                nc.vector.tensor_tensor(out=g3(tmpb[:]), in0=g3(tmpb[:]),
                                        in1=cslice(9 + kk), op=AL.add)
                nc.vector.tensor_tensor(out=p0[kk][:], in0=tmpa[:], in1=tmpb[:], op=AL.add)
            uu = tp.tile([128, SJ], f32t)
            vv = tp.tile([128, SJ], f32t)
            nc.vector.tensor_tensor(out=uu[:], in0=p0[0][:], in1=p0[2][:], op=AL.mult)
            nc.vector.tensor_tensor(out=vv[:], in0=p0[1][:], in1=p0[2][:], op=AL.mult)
            cofm = tp.tile([128, NBC * UJ * 12], f32t, tag="coeftab", name="cofm")
            nc.gpsimd.partition_broadcast(cofm[:], coefm1[:])
            coefm4 = cofm[:].rearrange("p (s j k) -> p s j k", j=UJ, k=12)

            def mslice(kidx):
                return coefm4[:, :, :, kidx]

            g = [tp.tile([128, SJ], f32t, name=f'g_{i}', tag=f'g_{i}') for i in range(3)]
            for kk in range(3):
                base = 3 * kk
                nc.vector.tensor_tensor(out=g3(tmpa[:]), in0=g3(uu[:]),
                                        in1=mslice(base + 0), op=AL.mult)
                nc.vector.tensor_tensor(out=g3(tmpb[:]), in0=g3(vv[:]),
                                        in1=mslice(base + 1), op=AL.mult)
                nc.vector.tensor_tensor(out=tmpa[:], in0=tmpa[:], in1=tmpb[:], op=AL.add)
                nc.vector.tensor_tensor(out=g3(tmpb[:]), in0=g3(p0[2][:]),
                                        in1=mslice(base + 2), op=AL.mult)
                nc.vector.tensor_tensor(out=tmpa[:], in0=tmpa[:], in1=tmpb[:], op=AL.add)
                nc.vector.tensor_tensor(out=g3(g[kk][:]), in0=g3(tmpa[:]),
                                        in1=mslice(9 + kk), op=AL.add)
            gx, gy, gz = g
            # ---- z-range mask (NZ=1): zm = (gz >= Lz0) * (gz < Lz1) ----
            zm = tp.tile([128, SJ], f32t)
            nc.vector.tensor_scalar(out=tmpa[:], in0=gz[:], scalar1=LZ0,
                                    scalar2=None, op0=AL.is_ge)
            nc.vector.tensor_scalar(out=tmpb[:], in0=gz[:], scalar1=LZ1,
                                    scalar2=None, op0=AL.is_lt)
            nc.vector.tensor_tensor(out=zm[:], in0=tmpa[:], in1=tmpb[:], op=AL.mult)

            gx4 = gx[:].rearrange("p (s j) -> p s j", j=UJ)
            gy4 = gy[:].rearrange("p (s j) -> p s j", j=UJ)
            zm4 = zm[:].rearrange("p (s j) -> p s j", j=UJ)

            # ---- batched exact binning: per-axis one-hot indicators ----
            WX1, WY1 = WXP + 1, WYP + 1
            cxa = tp.tile([128, NBC * UJ * WX1], bf16)
            cxa4 = cxa[:].rearrange("p (s j w) -> p s j w", j=UJ, w=WX1)
            nc.vector.tensor_tensor(
                out=cxa4,
                in0=gx4[:, :, :, None].broadcast_to([128, NBC, UJ, WX1]),
                in1=thrx[:].rearrange("p (s j w) -> p s j w", j=UJ, w=WX1),
                op=AL.is_ge)
            axa = tp.tile([128, NBC * UJ * WXP], bf16)
            axa4 = axa[:].rearrange("p (s j w) -> p s j w", j=UJ, w=WXP)
            nc.vector.tensor_tensor(out=axa4, in0=cxa4[:, :, :, 0:WXP],
                                    in1=cxa4[:, :, :, 1:WX1], op=AL.subtract)
            cya = tp.tile([128, NBC * UJ * WY1], bf16)
            cya4 = cya[:].rearrange("p (s j w) -> p s j w", j=UJ, w=WY1)
            nc.vector.tensor_tensor(
                out=cya4,
                in0=gy4[:, :, :, None].broadcast_to([128, NBC, UJ, WY1]),
                in1=thry[:].rearrange("p (s j w) -> p s j w", j=UJ, w=WY1),
                op=AL.is_ge)
            ayt = tp.tile([128, NBC * UJ * WYP], bf16)
            ayt4 = ayt[:].rearrange("p (s j w) -> p s j w", j=UJ, w=WYP)
            nc.vector.tensor_tensor(out=ayt4, in0=cya4[:, :, :, 0:WYP],
                                    in1=cya4[:, :, :, 1:WY1], op=AL.subtract)
            nc.vector.tensor_tensor(
                out=ayt4, in0=ayt4,
                in1=zm4[:, :, :, None].broadcast_to([128, NBC, UJ, WYP]),
                op=AL.mult)

            slots = tp.tile([C, NBC * BWP], bf16)

            # ---- per-block: one-hot outer product + matmul scatter ----
            PAIR = 4
            for s0 in range(0, NBC, PAIR):
                sl = min(PAIR, NBC - s0)
                fb = fp.tile([128, PAIR * UJ * C], bf16, tag="fb", name="fb")
                nc.sync.dma_start(
                    fb[:, :sl * UJ * C].rearrange("p (s x) -> p s x", x=UJ * C),
                    feats_t[s0:s0 + sl].rearrange("s (p j) c -> p s (j c)", p=128))
                for si in range(sl):
                    s = s0 + si
                    oh = op_.tile([128, UJ * WP], bf16, tag="oh", name="oh")
                    oh4 = oh[:].rearrange("p (j y x) -> p j y x", y=WYP, x=WXP)
                    nc.vector.tensor_tensor(
                        out=oh4,
                        in0=ayt4[:, s][:, :, :, None].broadcast_to([128, UJ, WYP, WXP]),
                        in1=axa4[:, s][:, :, None, :].broadcast_to([128, UJ, WYP, WXP]),
                        op=AL.mult)
                    JA = 6
                    psa = pp.tile([C, JA * WP], mybir.dt.float32, space="PSUM",
                                  tag="psa", name="psa")
                    psb = pp.tile([C, (UJ - JA) * WP], mybir.dt.float32, space="PSUM",
                                  tag="psb", name="psb")
                    for j in range(UJ):
                        dst = psa[:, j * WP:(j + 1) * WP] if j < JA else \
                            psb[:, (j - JA) * WP:(j - JA + 1) * WP]
                        nc.tensor.matmul(
                            dst,
                            lhsT=fb[:, (si * UJ + j) * C:(si * UJ + j + 1) * C],
                            rhs=oh[:, j * WP:(j + 1) * WP],
                            start=True, stop=True)
                    nc.scalar.copy(
                        out=slots[:, s * BWP:s * BWP + JA * WP], in_=psa[:])
                    nc.scalar.copy(
                        out=slots[:, s * BWP + JA * WP:(s + 1) * BWP], in_=psb[:])
                    if (s + 1) % 8 == 0 or s == NBC - 1:
                        lo = (s // 8) * 8
                        nc.sync.dma_start(slots_t[:, lo * BWP:(s + 1) * BWP],
                                          slots[:, lo * BWP:(s + 1) * BWP])

    nc.compile()
    return nc


_CACHE = {}


def kernel(**inputs) -> np.ndarray:
    from concourse.bass_utils import run_bass_kernel_spmd

    plan = _build_plan(inputs)
    NBC = plan['NBC']
    if NBC == 0:
        return np.zeros((B, C, NX, NY), f32)
    in_maps = [_pack_core(plan, inputs, c) for c in range(NCORES)]
    if NBC not in _CACHE:
        _CACHE.clear()
        _CACHE[NBC] = _build_bass(NBC)
    nc = _CACHE[NBC]

    r = run_bass_kernel_spmd(nc, in_maps, core_ids=list(range(NCORES)))
    out = np.zeros((B, C, NX, NY), f32)
    for c in range(NCORES):
        slots = r.results[c]['slots'].astype(f32).reshape(C, NBC, UJ, WYP, WXP)
        for s, blk in enumerate(plan['cores'][c]):
            for j, col in enumerate(blk['cols']):
                kx0, ky0 = col['kx0'], col['ky0']
                ex = min(WXP, NX - kx0); ey = min(WYP, NY - ky0)
                # slot layout [C, y, x]; output layout [C, X, Y]
                out[0, :, kx0:kx0 + ex, ky0:ky0 + ey] += \
                    slots[:, s, j, :ey, :ex].transpose(0, 2, 1)
    return out


# revision 22
# speedup vs baseline: 2.1410x; 2.1410x over previous
"""BEV camera-to-grid scatter kernel for Trainium2 (8 NeuronCores).

Strategy (BEVPoolV2-style):
 - Host (planning only): conservatively cull the 2M frustum points with f64
   geometry + margin (16.6% survive), sort kept points of each camera along a
   Morton curve of their target BEV cell, split contiguously across the 8
   cores, and pack into 1408-point blocks whose (margin-padded) cell windows
   fit a uniform WXP x WYP class. Ship features as bf16 plus small f32 tables
   (pixel coords, depth, per-block affine coefs, per-block exact f32 bin-edge
   thresholds).
 - Device (one uniform SPMD program, no control flow): batched f32 geometry
   for all points (exact reference op structure), exact binning via threshold
   compares diffed into per-axis one-hot indicators Ax/Ay, per-block one-hot
   outer products, bf16 matmul scatter into per-block PSUM windows, results
   appended to a per-block slots buffer, one DMA out. Cores are pure data
   parallel - no cross-core communication on device.
 - Host (unshard): place each block's window into the full (mostly zero) BEV
   grid and sum across cores (scatter-add is associative).
"""
import sys
import numpy as np

sys.path.insert(0, '/opt/trn_rl_repo')
import ml_dtypes

B, N, D, FH, FW, C = 1, 6, 118, 32, 88, 80
IH, IW = 256, 704
NX, NY, NZ = 360, 360, 1
DXS = (0.3, 0.3, 20.0)
COFF = (-54.0, -54.0, -10.0)   # exact f32 of reference's (bx - dx/2)
NCORES = 8
BLK = 1408                     # points per block: 128 partitions x 11 cols
UJ = 11
WXP, WYP = 8, 6                # uniform per-COLUMN window class (128 points)
WP = WXP * WYP
BWP = UJ * WP                  # slot cells per block
MARGIN_Q = 0.02                # conservative cull margin, in cell units
NCO = 24                       # per-block coefs: A(9) b(3) M(9) t(3)
PADTHR = 3.0e38
f32 = np.float32


# ---------------------------------------------------------------- thresholds
def _thresholds():
    """Exact f32 cell-edge thresholds replicating trunc((g-coff)/dx) binning.

    L[k] = smallest f32 g with q_of(g) >= k (k>=1); L[0] uses q_of(g) > -1
    (reference: trunc + coords>=0 keeps q in (-1,0) in bin 0).
    """
    out = []
    for ax, nb in ((0, NX), (1, NY), (2, NZ)):
        coff = f32(COFF[ax]); dx = f32(DXS[ax])

        def q_of(g):
            return f32(f32(f32(g) - coff) / dx)

        def smallest(pred, lo, hi):
            def key(i):
                return np.int64(i) if i >= 0 else np.int64(-2147483648) - np.int64(i)

            def unkey(k):
                return np.int32(k) if k >= 0 else np.int32(-(k + 2147483648))

            kl = key(f32(lo).view(np.int32)); kh = key(f32(hi).view(np.int32))
            assert not pred(unkey(kl).view(f32)) and pred(unkey(kh).view(f32))
            while kh - kl > 1:
                km = (kl + kh) // 2
                if pred(unkey(km).view(f32)):
                    kh = km
                else:
                    kl = km
            return unkey(kh).view(f32)

        lo_p = f32(coff - 4 * dx); hi_p = f32(coff + (nb + 4) * dx)
        L = np.empty(nb + 1, f32)
        L[0] = smallest(lambda g: q_of(g) > f32(-1.0), lo_p, hi_p)
        for k in range(1, nb + 1):
            L[k] = smallest(lambda g, k=k: q_of(g) >= f32(k), lo_p, hi_p)
        out.append(L)
    return out


_THR_CACHE = []


def _get_thresholds():
    if not _THR_CACHE:
        _THR_CACHE.append(_thresholds())
    return _THR_CACHE[0]


# ------------------------------------------------------------------- planning
def _frustum_axes():
    ds = np.arange(1.0, 60.0, 0.5, dtype=f32)
    xs = np.linspace(0.0, IW - 1, FW, dtype=f32)
    ys = np.linspace(0.0, IH - 1, FH, dtype=f32)
    return ds, xs, ys


def _compute_coeffs(inputs):
    """Fold the reference chain into per-cam affine A,b (pixel->p0) and M,t."""
    aug = np.asarray(inputs['img_aug_matrix'], np.float64)
    c2e = np.asarray(inputs['camera2ego'], np.float64)
    intr = np.asarray(inputs['camera_intrinsics'], np.float64)
    l2e = np.asarray(inputs['lidar2ego'], np.float64)
    laug = np.asarray(inputs['lidar_aug_matrix'], np.float64)
    inv_pr = np.linalg.inv(aug[..., :3, :3])
    post_trans = aug[..., :3, 3]
    A64 = inv_pr
    b64 = -np.einsum('bnij,bnj->bni', inv_pr, post_trans)
    combine = c2e[..., :3, :3] @ np.linalg.inv(intr[..., :3, :3])
    pre = laug[..., :3, :3] @ np.linalg.inv(l2e[..., :3, :3])
    M64 = np.einsum('bij,bnjk->bnik', pre, combine)
    t64 = np.einsum('bij,bnj->bni', pre, c2e[..., :3, 3] - l2e[..., :3, 3][:, None, :]) \
        + laug[..., :3, 3][:, None, :]
    return A64[0], b64[0], M64[0], t64[0]


def _geom64(A, b, M, t, px, py, dv):
    p0 = [A[k, 0] * px + A[k, 1] * py + (A[k, 2] * dv + b[k]) for k in range(3)]
    uu = p0[0] * p0[2]
    vv = p0[1] * p0[2]
    return [(uu * M[k, 0] + vv * M[k, 1]) + p0[2] * M[k, 2] + t[k] for k in range(3)]


def _build_plan(inputs):
    A64, b64, M64, t64 = _compute_coeffs(inputs)
    ds, xs, ys = _frustum_axes()
    dvg, pyg, pxg = np.meshgrid(ds.astype(np.float64), ys.astype(np.float64),
                                xs.astype(np.float64), indexing='ij')
    pxg = np.ascontiguousarray(pxg.ravel())
    pyg = np.ascontiguousarray(pyg.ravel())
    dvg = np.ascontiguousarray(dvg.ravel())
    colslists = [[] for _ in range(NCORES)]
    for n in range(N):
        gx, gy, gz = _geom64(A64[n], b64[n], M64[n], t64[n], pxg, pyg, dvg)
        qx = (gx - COFF[0]) / DXS[0]
        qy = (gy - COFF[1]) / DXS[1]
        qz = (gz - COFF[2]) / DXS[2]
        m = MARGIN_Q
        keep = ((qx > -1 - m) & (qx < NX + m) &
                (qy > -1 - m) & (qy < NY + m) &
                (qz > -1 - m) & (qz < NZ + m))
        idx = np.nonzero(keep)[0]
        kx = np.maximum(np.floor(qx[idx]), 0).astype(np.int64)
        ky = np.maximum(np.floor(qy[idx]), 0).astype(np.int64)
        code = np.zeros(len(kx), np.int64)
        for bit in range(9):
            code |= ((kx >> bit) & 1) << (2 * bit) | ((ky >> bit) & 1) << (2 * bit + 1)
        order = np.argsort(code, kind='stable')
        idx, kx, ky = idx[order], kx[order], ky[order]
        K = len(idx)
        i = 0
        cols = []
        while i < K:
            j = i
            x0 = x1 = kx[i]; y0 = y1 = ky[i]
            while j < K and j - i < 128:
                nx0 = min(x0, kx[j]); nx1 = max(x1, kx[j])
                ny0 = min(y0, ky[j]); ny1 = max(y1, ky[j])
                if nx1 - nx0 + 3 > WXP or ny1 - ny0 + 3 > WYP:
                    break
                x0, x1, y0, y1 = nx0, nx1, ny0, ny1
                j += 1
            cols.append(dict(idx=idx[i:j], cam=n,
                             kx0=max(int(x0) - 1, 0),
                             ky0=max(int(y0) - 1, 0)))
            i = j
        # deal columns round-robin so per-core counts stay balanced
        for ci, col in enumerate(cols):
            colslists[ci % NCORES].append(col)
    # group 11 columns per block (cameras may mix: coefs are per column)
    cores = [[dict(cols=cl[b0:b0 + UJ]) for b0 in range(0, len(cl), UJ)]
             for cl in colslists]
    NBC = max(len(c) for c in cores)
    return dict(A64=A64, b64=b64, M64=M64, t64=t64, cores=cores, NBC=NBC,
                pxg=pxg, pyg=pyg, dvg=dvg)


def _pack_core(plan, inputs, c):
    """Device-side tables for core c."""
    Lx, Ly, Lz = _get_thresholds()
    NBC = plan['NBC']
    cf = np.asarray(inputs['cam_feats'], f32)[0].reshape(N, -1, C)
    blocks = plan['cores'][c]
    feats = np.zeros((NBC, BLK, C), ml_dtypes.bfloat16)
    pxt = np.zeros((128, UJ * NBC), f32)
    pyt = np.zeros((128, UJ * NBC), f32)
    dvt = np.zeros((128, UJ * NBC), f32)
    coef = np.zeros((NBC, UJ, NCO), f32)
    thrx = np.full((NBC, UJ, WXP + 1), PADTHR, f32)
    thry = np.full((NBC, UJ, WYP + 1), PADTHR, f32)
    for s, blk in enumerate(blocks):
        fe = feats[s].reshape(128, UJ, C)
        for j, col in enumerate(blk['cols']):
            n = col['cam']
            A = plan['A64'][n].astype(f32); b = plan['b64'][n].astype(f32)
            M = plan['M64'][n].astype(f32); t = plan['t64'][n].astype(f32)
            coef[s, j] = np.array(list(A.ravel()) + list(b) + list(M.ravel())
                                  + list(t), f32)
            idx = col['idx']
            k = len(idx)
            fe[:k, j] = cf[n][idx].astype(ml_dtypes.bfloat16)
            pxt[:k, s * UJ + j] = plan['pxg'][idx]
            pyt[:k, s * UJ + j] = plan['pyg'][idx]
            dvt[:k, s * UJ + j] = plan['dvg'][idx]
            ex = min(WXP + 1, NX + 1 - col['kx0'])
            ey = min(WYP + 1, NY + 1 - col['ky0'])
            thrx[s, j, :ex] = Lx[col['kx0']:col['kx0'] + ex]
            thry[s, j, :ey] = Ly[col['ky0']:col['ky0'] + ey]
    coefa = np.ascontiguousarray(coef[..., :12]).reshape(1, NBC * UJ * 12)
    coefm = np.ascontiguousarray(coef[..., 12:]).reshape(1, NBC * UJ * 12)
    thrxb = thrx.reshape(1, NBC * UJ * (WXP + 1)).copy()
    thryb = thry.reshape(1, NBC * UJ * (WYP + 1)).copy()
    return dict(feats=feats, pxt=pxt, pyt=pyt, dvt=dvt, coefa=coefa,
                coefm=coefm, thrx=thrxb, thry=thryb)


# ----------------------------------------------------------------- bass build
def _build_bass(NBC):
    import concourse.bacc as bacc
    import concourse.mybir as mybir
    import concourse.tile as tile

    SJ = NBC * UJ
    f32t = mybir.dt.float32
    bf16 = mybir.dt.bfloat16
    AL = mybir.AluOpType
    Lx, Ly, Lz = _get_thresholds()
    LZ0, LZ1 = float(Lz[0]), float(Lz[1])

    nc = bacc.Bacc(None, target_bir_lowering=False, num_devices=NCORES)
    feats_t = nc.dram_tensor("feats", [NBC, BLK, C], bf16, kind="ExternalInput")
    pxt_t = nc.dram_tensor("pxt", [128, SJ], f32t, kind="ExternalInput")
    pyt_t = nc.dram_tensor("pyt", [128, SJ], f32t, kind="ExternalInput")
    dvt_t = nc.dram_tensor("dvt", [128, SJ], f32t, kind="ExternalInput")
    coefa_t = nc.dram_tensor("coefa", [1, NBC * UJ * 12], f32t, kind="ExternalInput")
    coefm_t = nc.dram_tensor("coefm", [1, NBC * UJ * 12], f32t, kind="ExternalInput")
    thrx_t = nc.dram_tensor("thrx", [1, NBC * UJ * (WXP + 1)], f32t,
                            kind="ExternalInput")
    thry_t = nc.dram_tensor("thry", [1, NBC * UJ * (WYP + 1)], f32t,
                            kind="ExternalInput")
    slots_t = nc.dram_tensor("slots", [C, NBC * BWP], bf16, kind="ExternalOutput")

    with tile.TileContext(nc) as tc:
        with tc.tile_pool(name="tabs", bufs=1) as tp, \
             tc.tile_pool(name="fb", bufs=2) as fp, \
             tc.tile_pool(name="oh", bufs=4) as op_, \
             tc.tile_pool(name="ps", bufs=4, space="PSUM") as pp:

            pxt = tp.tile([128, SJ], f32t); nc.sync.dma_start(pxt[:], pxt_t[:])
            pyt = tp.tile([128, SJ], f32t); nc.sync.dma_start(pyt[:], pyt_t[:])
            dvt = tp.tile([128, SJ], f32t); nc.sync.dma_start(dvt[:], dvt_t[:])
            coefa1 = tp.tile([1, NBC * UJ * 12], f32t)
            nc.sync.dma_start(coefa1[:], coefa_t[:])
            coefm1 = tp.tile([1, NBC * UJ * 12], f32t)
            nc.sync.dma_start(coefm1[:], coefm_t[:])
            thrx1 = tp.tile([1, NBC * UJ * (WXP + 1)], f32t)
            nc.sync.dma_start(thrx1[:], thrx_t[:])
            thry1 = tp.tile([1, NBC * UJ * (WYP + 1)], f32t)
            nc.sync.dma_start(thry1[:], thry_t[:])
            cofa = tp.tile([128, NBC * UJ * 12], f32t, tag="coeftab", name="cofa")
            nc.gpsimd.partition_broadcast(cofa[:], coefa1[:])
            thrx = tp.tile([128, NBC * UJ * (WXP + 1)], f32t)
            nc.gpsimd.partition_broadcast(thrx[:], thrx1[:])
            thry = tp.tile([128, NBC * UJ * (WYP + 1)], f32t)
            nc.gpsimd.partition_broadcast(thry[:], thry1[:])

            coefa4 = cofa[:].rearrange("p (s j k) -> p s j k", j=UJ, k=12)

            def cslice(kidx):
                return coefa4[:, :, :, kidx]

            def g3(ap):
                return ap.rearrange("p (s j) -> p s j", j=UJ)

            # ---- batched geometry, exact f32 op order ----
            tmpa = tp.tile([128, SJ], f32t)
            tmpb = tp.tile([128, SJ], f32t)
            p0 = [tp.tile([128, SJ], f32t, name=f'p0_{i}', tag=f'p0_{i}')
                  for i in range(3)]
            for kk in range(3):
                nc.vector.tensor_tensor(out=g3(tmpa[:]), in0=g3(pxt[:]),
                                        in1=cslice(3 * kk + 0), op=AL.mult)
                nc.vector.tensor_tensor(out=g3(tmpb[:]), in0=g3(pyt[:]),
                                        in1=cslice(3 * kk + 1), op=AL.mult)
                nc.vector.tensor_tensor(out=tmpa[:], in0=tmpa[:], in1=tmpb[:], op=AL.add)
                nc.vector.tensor_tensor(out=g3(tmpb[:]), in0=g3(dvt[:]),
                                        in1=cslice(3 * kk + 2), op=AL.mult)
                nc.vector.tensor_tensor(out=g3(tmpb[:]), in0=g3(tmpb[:]),
                                        in1=cslice(9 + kk), op=AL.add)
                nc.vector.tensor_tensor(out=p0[kk][:], in0=tmpa[:], in1=tmpb[:], op=AL.add)
            uu = tp.tile([128, SJ], f32t)
            vv = tp.tile([128, SJ], f32t)
            nc.vector.tensor_tensor(out=uu[:], in0=p0[0][:], in1=p0[2][:], op=AL.mult)
            nc.vector.tensor_tensor(out=vv[:], in0=p0[1][:], in1=p0[2][:], op=AL.mult)
            cofm = tp.tile([128, NBC * UJ * 12], f32t, tag="coeftab", name="cofm")
            nc.gpsimd.partition_broadcast(cofm[:], coefm1[:])
            coefm4 = cofm[:].rearrange("p (s j k) -> p s j k", j=UJ, k=12)

            def mslice(kidx):
                return coefm4[:, :, :, kidx]

            g = [tp.tile([128, SJ], f32t, name=f'g_{i}', tag=f'g_{i}') for i in range(3)]
            for kk in range(3):
                base = 3 * kk
                nc.vector.tensor_tensor(out=g3(tmpa[:]), in0=g3(uu[:]),
                                        in1=mslice(base + 0), op=AL.mult)
                nc.vector.tensor_tensor(out=g3(tmpb[:]), in0=g3(vv[:]),
                                        in1=mslice(base + 1), op=AL.mult)
                nc.vector.tensor_tensor(out=tmpa[:], in0=tmpa[:], in1=tmpb[:], op=AL.add)
                nc.vector.tensor_tensor(out=g3(tmpb[:]), in0=g3(p0[2][:]),
                                        in1=mslice(base + 2), op=AL.mult)
                nc.vector.tensor_tensor(out=tmpa[:], in0=tmpa[:], in1=tmpb[:], op=AL.add)
                nc.vector.tensor_tensor(out=g3(g[kk][:]), in0=g3(tmpa[:]),
                                        in1=mslice(9 + kk), op=AL.add)
            gx, gy, gz = g
            # ---- z-range mask (NZ=1): zm = (gz >= Lz0) * (gz < Lz1) ----
            zm = tp.tile([128, SJ], f32t)
            nc.vector.tensor_scalar(out=tmpa[:], in0=gz[:], scalar1=LZ0,
                                    scalar2=None, op0=AL.is_ge)
            nc.vector.tensor_scalar(out=tmpb[:], in0=gz[:], scalar1=LZ1,
                                    scalar2=None, op0=AL.is_lt)
            nc.vector.tensor_tensor(out=zm[:], in0=tmpa[:], in1=tmpb[:], op=AL.mult)

            gx4 = gx[:].rearrange("p (s j) -> p s j", j=UJ)
            gy4 = gy[:].rearrange("p (s j) -> p s j", j=UJ)
            zm4 = zm[:].rearrange("p (s j) -> p s j", j=UJ)

            # ---- batched exact binning: per-axis one-hot indicators ----
            WX1, WY1 = WXP + 1, WYP + 1
            cxa = tp.tile([128, NBC * UJ * WX1], bf16)
            cxa4 = cxa[:].rearrange("p (s j w) -> p s j w", j=UJ, w=WX1)
            nc.vector.tensor_tensor(
                out=cxa4,
                in0=gx4[:, :, :, None].broadcast_to([128, NBC, UJ, WX1]),
                in1=thrx[:].rearrange("p (s j w) -> p s j w", j=UJ, w=WX1),
                op=AL.is_ge)
            axa = tp.tile([128, NBC * UJ * WXP], bf16)
            axa4 = axa[:].rearrange("p (s j w) -> p s j w", j=UJ, w=WXP)
            nc.vector.tensor_tensor(out=axa4, in0=cxa4[:, :, :, 0:WXP],
                                    in1=cxa4[:, :, :, 1:WX1], op=AL.subtract)
            cya = tp.tile([128, NBC * UJ * WY1], bf16)
            cya4 = cya[:].rearrange("p (s j w) -> p s j w", j=UJ, w=WY1)
            nc.vector.tensor_tensor(
                out=cya4,
                in0=gy4[:, :, :, None].broadcast_to([128, NBC, UJ, WY1]),
                in1=thry[:].rearrange("p (s j w) -> p s j w", j=UJ, w=WY1),
                op=AL.is_ge)
            ayt = tp.tile([128, NBC * UJ * WYP], bf16)
            ayt4 = ayt[:].rearrange("p (s j w) -> p s j w", j=UJ, w=WYP)
            nc.vector.tensor_tensor(out=ayt4, in0=cya4[:, :, :, 0:WYP],
                                    in1=cya4[:, :, :, 1:WY1], op=AL.subtract)
            nc.vector.tensor_tensor(
                out=ayt4, in0=ayt4,
                in1=zm4[:, :, :, None].broadcast_to([128, NBC, UJ, WYP]),
                op=AL.mult)

            slots = tp.tile([C, NBC * BWP], bf16)

            # ---- per-block: one-hot outer product + matmul scatter ----
            PAIR = 4
            for s0 in range(0, NBC, PAIR):
                sl = min(PAIR, NBC - s0)
                fb = fp.tile([128, PAIR * UJ * C], bf16, tag="fb", name="fb")
                nc.sync.dma_start(
                    fb[:, :sl * UJ * C].rearrange("p (s x) -> p s x", x=UJ * C),
                    feats_t[s0:s0 + sl].rearrange("s (p j) c -> p s (j c)", p=128))
                for si in range(sl):
                    s = s0 + si
                    oh = op_.tile([128, UJ * WP], bf16, tag="oh", name="oh")
                    oh4 = oh[:].rearrange("p (j y x) -> p j y x", y=WYP, x=WXP)
                    nc.vector.tensor_tensor(
                        out=oh4,
                        in0=ayt4[:, s][:, :, :, None].broadcast_to([128, UJ, WYP, WXP]),
                        in1=axa4[:, s][:, :, None, :].broadcast_to([128, UJ, WYP, WXP]),
                        op=AL.mult)
                    JA = 6
                    psa = pp.tile([C, JA * WP], mybir.dt.float32, space="PSUM",
                                  tag="psa", name="psa")
                    psb = pp.tile([C, (UJ - JA) * WP], mybir.dt.float32, space="PSUM",
                                  tag="psb", name="psb")
                    for j in range(UJ):
                        dst = psa[:, j * WP:(j + 1) * WP] if j < JA else \
                            psb[:, (j - JA) * WP:(j - JA + 1) * WP]
                        nc.tensor.matmul(
                            dst,
                            lhsT=fb[:, (si * UJ + j) * C:(si * UJ + j + 1) * C],
                            rhs=oh[:, j * WP:(j + 1) * WP],
                            start=True, stop=True)
                    nc.scalar.copy(
                        out=slots[:, s * BWP:s * BWP + JA * WP], in_=psa[:])
                    nc.scalar.copy(
                        out=slots[:, s * BWP + JA * WP:(s + 1) * BWP], in_=psb[:])
                    if (s + 1) % 8 == 0 or s == NBC - 1:
                        lo = (s // 8) * 8
                        nc.sync.dma_start(slots_t[:, lo * BWP:(s + 1) * BWP],
                                          slots[:, lo * BWP:(s + 1) * BWP])

    nc.compile()
    return nc


_CACHE = {}


def kernel(**inputs) -> np.ndarray:
    from concourse.bass_utils import run_bass_kernel_spmd

    plan = _build_plan(inputs)
    NBC = plan['NBC']
    if NBC == 0:
        return np.zeros((B, C, NX, NY), f32)
    in_maps = [_pack_core(plan, inputs, c) for c in range(NCORES)]
    if NBC not in _CACHE:
        _CACHE.clear()
        _CACHE[NBC] = _build_bass(NBC)
    nc = _CACHE[NBC]

    r = run_bass_kernel_spmd(nc, in_maps, core_ids=list(range(NCORES)))
    out = np.zeros((B, C, NX, NY), f32)
    for c in range(NCORES):
        slots = r.results[c]['slots'].astype(f32).reshape(C, NBC, UJ, WYP, WXP)
        for s, blk in enumerate(plan['cores'][c]):
            for j, col in enumerate(blk['cols']):
                kx0, ky0 = col['kx0'], col['ky0']
                ex = min(WXP, NX - kx0); ey = min(WYP, NY - ky0)
                # slot layout [C, y, x]; output layout [C, X, Y]
                out[0, :, kx0:kx0 + ex, ky0:ky0 + ey] += \
                    slots[:, s, j, :ey, :ex].transpose(0, 2, 1)
    return out
